# revision 1
# baseline (speedup 1.0000x reference)
"""Trainium2 Bass kernel for CalculateInstanceSize (segment_reduce).

Contract: kernel(seg_outs, pad_ins_outs) -> [B, N, 3] float32, matching
the jax reference. B=8 batches are data-parallel across the 8 NeuronCores;
each core computes its batch's per-row regression (unit length) and the
three weighted reductions over pad [N, H, W].

Layout notes:
- The per-row regression runs in "column space" [128, NCH] (h = c*128+p),
  so the h-cumsum is a triangular matmul and the weighted sums are a
  ones-vector matmul landing in a [1,7] PSUM row. No transposes needed.
- pad ships as fp16 (halves DMA; error ~2^-11/sqrt(N) after fp32
  accumulation), seg as bf16 (only its sign pattern matters: pos = seg>0).
- occ row-counts are spread over DVE/ACT/Pool so no single engine is the
  bottleneck; occ = count > 0 for all three formulations.
- Engine APs must start at partition 0 (walrus rule); only DMAs may read
  other partition offsets, which the final output DMAs rely on.
"""

import sys

sys.path.insert(0, "/opt/trn_rl_repo")

import numpy as np

import concourse.bass as bass
import concourse.tile as tile
from concourse import bacc, mybir
from concourse.bass_utils import run_bass_kernel_spmd

F32 = mybir.dt.float32
F16 = mybir.dt.float16
BF16 = mybir.dt.bfloat16
AX = mybir.AxisListType
OP = mybir.AluOpType
ACTF = mybir.ActivationFunctionType

B, H, W, N = 8, 512, 512, 32
NCH = H // 128  # h-chunks of 128 partitions
ROAD = 3.25


def build_kernel(reps: int = 1):
    nc = bacc.Bacc("TRN2", target_bir_lowering=False, debug=False, num_devices=B)

    seg = nc.dram_tensor("seg", [128, NCH, W], BF16, kind="ExternalInput").ap()
    pad = nc.dram_tensor("pad", [N, H, W], F16, kind="ExternalInput").ap()
    yf = nc.dram_tensor("yf", [128, NCH], F32, kind="ExternalInput").ap()
    tril = nc.dram_tensor("tril", [128, 128], F32, kind="ExternalInput").ap()
    amin4 = nc.dram_tensor("amin4", [128, NCH, W], F16, kind="ExternalInput").ap()
    amax4 = nc.dram_tensor("amax4", [128, NCH, W], F16, kind="ExternalInput").ap()
    out = nc.dram_tensor("out", [3, N], F32, kind="ExternalOutput").ap()

    with tile.TileContext(nc) as tc:
        emit(tc, out, seg, pad, yf, tril, amin4, amax4, reps)
    nc.compile()
    return nc


def emit(tc, out, seg, pad, yf, tril, amin4, amax4, reps=1):
    nc = tc.nc
    import contextlib

    ctx = contextlib.ExitStack()
    with ctx:
        consts = ctx.enter_context(tc.tile_pool(name="consts", bufs=1))
        padp = ctx.enter_context(tc.tile_pool(name="padp", bufs=16))
        evacp = ctx.enter_context(tc.tile_pool(name="evacp", bufs=6))
        psp = ctx.enter_context(tc.psum_pool(name="psp", bufs=5))
        psv = ctx.enter_context(tc.psum_pool(name="psv", bufs=1))
        pss = ctx.enter_context(tc.psum_pool(name="pss", bufs=1))

        # ---- prologue inputs (seg first: it heads the critical path) ----
        SEGB = consts.tile([128, NCH, W], BF16)
        nc.sync.dma_start(SEGB[:], seg[:])
        AMIN4 = consts.tile([128, NCH, W], F16)
        nc.sync.dma_start(AMIN4[:], amin4[:])
        AMAX4 = consts.tile([128, NCH, W], F16)
        nc.sync.dma_start(AMAX4[:], amax4[:])
        YF = consts.tile([128, NCH], F32)
        nc.sync.dma_start(YF[:], yf[:])
        TRIL = consts.tile([128, 128], F32)
        nc.sync.dma_start(TRIL[:], tril[:])
        ONES1 = consts.tile([128, 1], F32)
        nc.gpsimd.memset(ONES1[:], 1.0)
        NEGH = consts.tile([128, 1], F32)
        nc.gpsimd.memset(NEGH[:], -0.5)

        # ---- per-row x_min / x_max in column space ----
        # mask = seg > 0; R0 = max_w (W-w)*m -> xmin = W - R0
        #                 R1 = max_w (w+1)*m -> xmax = R1 - 1
        MSK = consts.tile([128, NCH, W], F16)
        nc.vector.tensor_scalar(
            out=MSK[:], in0=SEGB[:], scalar1=0.0, scalar2=None, op0=OP.is_gt
        )
        TMIN = consts.tile([128, NCH, W], F16)
        nc.vector.tensor_tensor(out=TMIN[:], in0=MSK[:], in1=AMIN4[:], op=OP.mult)
        TMAX = consts.tile([128, NCH, W], F16)
        nc.vector.tensor_tensor(out=TMAX[:], in0=MSK[:], in1=AMAX4[:], op=OP.mult)
        R0 = consts.tile([128, NCH], F32)
        nc.vector.tensor_reduce(out=R0[:], in_=TMIN[:], axis=AX.X, op=OP.max)
        R1 = consts.tile([128, NCH], F32)
        nc.vector.tensor_reduce(out=R1[:], in_=TMAX[:], axis=AX.X, op=OP.max)
        XMIN4 = consts.tile([128, NCH], F32)
        nc.vector.tensor_scalar(
            out=XMIN4[:], in0=R0[:], scalar1=-1.0, scalar2=float(W), op0=OP.mult,
            op1=OP.add,
        )
        XMAX4 = consts.tile([128, NCH], F32)
        nc.vector.tensor_scalar(
            out=XMAX4[:], in0=R1[:], scalar1=1.0, scalar2=None, op0=OP.subtract
        )

        # ---- validity + rank (global h-cumsum via triangular matmul) ----
        NE4 = consts.tile([128, NCH], F32)
        nc.vector.tensor_tensor(out=NE4[:], in0=XMIN4[:], in1=XMAX4[:], op=OP.not_equal)
        V4 = consts.tile([128, NCH], F32)
        nc.vector.scalar_tensor_tensor(
            out=V4[:], in0=XMAX4[:], scalar=-0.5, in1=NE4[:], op0=OP.is_gt, op1=OP.mult
        )
        CUM4 = pss.tile([128, NCH], F32, tag="cum4")
        nc.tensor.matmul(out=CUM4[:], lhsT=TRIL[:], rhs=V4[:], start=True, stop=True)
        CS = pss.tile([1, NCH], F32, tag="small")
        nc.tensor.matmul(out=CS[:], lhsT=ONES1[:], rhs=V4[:], start=True, stop=True)
        # exclusive prefix of per-column sums
        OFFS = consts.tile([1, NCH], F32)
        nc.vector.memset(OFFS[:], 0.0)
        nc.vector.tensor_copy(OFFS[0:1, 1:NCH], CS[0:1, 0 : NCH - 1])
        nc.vector.tensor_tensor(
            out=OFFS[0:1, 2:NCH], in0=OFFS[0:1, 2:NCH], in1=OFFS[0:1, 0 : NCH - 2],
            op=OP.add,
        )
        # scalars packed into SCP = [t, t-1, n_valid, 0]
        SCP = consts.tile([1, NCH], F32)
        NV = SCP[0:1, 2:3]
        nc.vector.tensor_reduce(out=NV, in_=CS[:], axis=AX.X, op=OP.add)
        TVv = SCP[0:1, 0:1]
        nc.vector.tensor_scalar(
            out=TVv, in0=NV, scalar1=0.15, scalar2=None, op0=OP.mult
        )
        nc.vector.tensor_scalar(
            out=SCP[0:1, 1:2], in0=TVv, scalar1=1.0, scalar2=None, op0=OP.subtract
        )
        nc.vector.memset(SCP[0:1, 3:4], 0.0)
        SCB = consts.tile([128, NCH], F32)
        nc.gpsimd.partition_broadcast(SCB[:], SCP[0:1, :])
        OFFSB = consts.tile([128, NCH], F32)
        nc.gpsimd.partition_broadcast(OFFSB[:], OFFS[0:1, :])
        RANK4 = consts.tile([128, NCH], F32)
        nc.vector.scalar_tensor_tensor(
            out=RANK4[:], in0=CUM4[:], scalar=-1.0, in1=OFFSB[:], op0=OP.add,
            op1=OP.add,
        )
        # keep = valid & rank>t-1 & rank>=1 & (n-rank)>t & (n-rank)>1.5
        M4 = consts.tile([128, NCH], F32)
        nc.vector.tensor_scalar(
            out=M4[:], in0=RANK4[:], scalar1=SCB[:, 2:3], scalar2=-1.0,
            op0=OP.subtract, op1=OP.mult,
        )
        K1 = consts.tile([128, NCH], F32)
        nc.vector.scalar_tensor_tensor(
            out=K1[:], in0=RANK4[:], scalar=SCB[:, 1:2], in1=V4[:], op0=OP.is_gt,
            op1=OP.mult,
        )
        K2 = consts.tile([128, NCH], F32)
        nc.vector.scalar_tensor_tensor(
            out=K2[:], in0=RANK4[:], scalar=0.5, in1=K1[:], op0=OP.is_gt, op1=OP.mult
        )
        K3 = consts.tile([128, NCH], F32)
        nc.vector.scalar_tensor_tensor(
            out=K3[:], in0=M4[:], scalar=SCB[:, 0:1], in1=K2[:], op0=OP.is_gt,
            op1=OP.mult,
        )
        W4 = consts.tile([128, NCH], F32)
        nc.vector.scalar_tensor_tensor(
            out=W4[:], in0=M4[:], scalar=1.5, in1=K3[:], op0=OP.is_gt, op1=OP.mult
        )

        # ---- weighted sums S = [Sw, Sy, Syy, SxL, SxyL, SxR, SxyR] ----
        # (ones-matmul over the h-partitions; all addends here are integers
        # so the PE's decomposed fp32 multiply is exact)
        S7 = consts.tile([128, NCH, 7], F32)
        nc.vector.tensor_copy(S7[:, :, 0], W4[:])
        nc.vector.tensor_tensor(out=S7[:, :, 1], in0=W4[:], in1=YF[:], op=OP.mult)
        nc.vector.tensor_tensor(out=S7[:, :, 2], in0=S7[:, :, 1], in1=YF[:], op=OP.mult)
        nc.vector.tensor_tensor(out=S7[:, :, 3], in0=W4[:], in1=XMIN4[:], op=OP.mult)
        nc.vector.tensor_tensor(out=S7[:, :, 4], in0=S7[:, :, 3], in1=YF[:], op=OP.mult)
        nc.vector.tensor_tensor(out=S7[:, :, 5], in0=W4[:], in1=XMAX4[:], op=OP.mult)
        nc.vector.tensor_tensor(out=S7[:, :, 6], in0=S7[:, :, 5], in1=YF[:], op=OP.mult)
        SS = pss.tile([1, 7], F32, tag="small")
        for c in range(NCH):
            nc.tensor.matmul(
                out=SS[:], lhsT=ONES1[:], rhs=S7[:, c, :], start=(c == 0),
                stop=(c == NCH - 1),
            )

        # ---- 2x2 normal-equation solve, batched on [1,k] rows ----
        # G pairs (even*odd): (0,1)=(Sw*SxyL, Sy*SxL)  (2,3)=(Syy*SxL, Sy*SxyL)
        #                     (4,5)=(Sw*SxyR, Sy*SxR)  (6,7)=(Syy*SxR, Sy*SxyR)
        #                     (8,9)=(Syy*Sw, Sy*Sy)
        # D[0:5] = G[even] - G[odd] = [nsL, niL, nsR, niR, det]
        G = consts.tile([1, 10], F32)
        SR = consts.tile([1, 7], F32)
        nc.vector.tensor_copy(SR[:], SS[:])  # PSUM -> SBUF (TT can't read 2x PSUM)

        # strided pair products out of the [1,7] sums row
        def pair(dst0, a0, a1):
            nc.vector.tensor_tensor(
                out=G[0:1, dst0 : dst0 + 2], in0=a0, in1=a1, op=OP.mult
            )

        up01 = SR[0:1, 0:2]  # (Sw, Sy)
        dn21 = SR[0:1, 2:0:-1]  # (Syy, Sy)
        pair(0, up01, SR[0:1, 4:2:-1])  # (Sw*SxyL, Sy*SxL)
        pair(2, dn21, SR[0:1, 3:5])  # (Syy*SxL, Sy*SxyL)
        pair(4, up01, SR[0:1, 6:4:-1])  # (Sw*SxyR, Sy*SxR)
        pair(6, dn21, SR[0:1, 5:7])  # (Syy*SxR, Sy*SxyR)
        pair(8, dn21, up01)  # (Syy*Sw, Sy*Sy)
        D = consts.tile([1, 8], F32)
        nc.vector.tensor_tensor(
            out=D[0:1, 0:5], in0=G[0:1, 0:10:2], in1=G[0:1, 1:10:2], op=OP.subtract
        )
        DET = D[0:1, 4:5]
        OKV = D[0:1, 5:6]
        nc.vector.tensor_scalar(
            out=OKV, in0=DET, scalar1=0.0, scalar2=None, op0=OP.is_gt
        )
        # safe = det*ok + (1-ok); rsafe = 1/safe
        SAFE = D[0:1, 6:7]
        nc.vector.scalar_tensor_tensor(
            out=SAFE, in0=DET, scalar=1.0, in1=OKV, op0=OP.subtract, op1=OP.mult
        )  # (det-1)*ok
        nc.vector.tensor_scalar(
            out=SAFE, in0=SAFE, scalar1=1.0, scalar2=None, op0=OP.add
        )  # (det-1)*ok + 1 = det*ok + (1-ok)
        RS = D[0:1, 7:8]
        nc.vector.reciprocal(out=RS, in_=SAFE)
        SLIC = consts.tile([1, NCH], F32)
        nc.vector.tensor_scalar(
            out=SLIC[:], in0=D[0:1, 0:4], scalar1=RS, scalar2=OKV, op0=OP.mult,
            op1=OP.mult,
        )

        # ---- unit / unit^2 weights ----
        SB = consts.tile([128, NCH], F32)
        nc.gpsimd.partition_broadcast(SB[:], SLIC[0:1, :])
        PRL = consts.tile([128, NCH], F32)
        nc.vector.tensor_scalar(
            out=PRL[:], in0=YF[:], scalar1=SB[:, 0:1], scalar2=SB[:, 1:2],
            op0=OP.mult, op1=OP.add,
        )
        PRR = consts.tile([128, NCH], F32)
        nc.vector.tensor_scalar(
            out=PRR[:], in0=YF[:], scalar1=SB[:, 2:3], scalar2=SB[:, 3:4],
            op0=OP.mult, op1=OP.add,
        )
        WID = consts.tile([128, NCH], F32)
        nc.vector.tensor_tensor(out=WID[:], in0=PRR[:], in1=PRL[:], op=OP.subtract)
        nc.vector.tensor_scalar(
            out=WID[:], in0=WID[:], scalar1=1.0, scalar2=None, op0=OP.max
        )
        RCP = consts.tile([128, NCH], F32)
        nc.vector.reciprocal(out=RCP[:], in_=WID[:])
        UU = consts.tile([128, NCH, 2], F32)
        nc.vector.tensor_scalar(
            out=UU[:, :, 0], in0=RCP[:], scalar1=ROAD, scalar2=None, op0=OP.mult
        )
        nc.vector.scalar_tensor_tensor(
            out=UU[:, :, 1], in0=RCP[:], scalar=ROAD * ROAD, in1=RCP[:],
            op0=OP.mult, op1=OP.mult,
        )
        UUH = consts.tile([128, NCH, 2], F16)
        nc.vector.tensor_copy(UUH[:], UU[:])
        
        # ---- main loop over instances ----
        for _rep in range(reps):
            CNT = consts.tile([128, NCH, N], F32)  # per (h, c, n): #(pad > 0.5) in row
            # psum row 0 = T[w] = sum_h unit*pad ; row 1 = U2[w] = sum_h unit2*pad
            HORP = consts.tile([2, N], F32)  # row 0 = max_w T  (row 1 junk)
            INSTP = consts.tile([2, N], F32)  # row 1 = sum_w U2 (row 0 junk)
            JD = consts.tile([128, W], F16)
            JA = consts.tile([128, W], F16)
            JP = consts.tile([128, W], F16)
            shares = {"D": 85, "A": 43, "P": 0}
            assign, used = [], {k: 0 for k in shares}
            for i in range(N * NCH):
                k = max(shares, key=lambda e: (i + 1) * shares[e] / 128 - used[e])
                used[k] += 1
                assign.append(k)
            padr = pad.rearrange("n (c p) w -> n p c w", p=128)
            for n in range(N):
                PS = psp.tile([2, W], F32, tag="ps")
                PT = padp.tile([128, NCH, W], F16, tag="pt")
                nc.sync.dma_start(PT[:], padr[n])
                for c in range(NCH):
                    nc.tensor.matmul(
                        out=PS[:],
                        lhsT=UUH[:, c, :],
                        rhs=PT[:, c, :],
                        start=(c == 0),
                        stop=(c == NCH - 1),
                    )
                    eng = assign[n * NCH + c]
                    if eng == "D":
                        nc.vector.tensor_scalar(
                            out=JD[:], in0=PT[:, c, :], scalar1=0.5, scalar2=None,
                            op0=OP.is_gt, op1=OP.add, accum_out=CNT[:, c, n : n + 1],
                        )
                    elif eng == "P":
                        nc.gpsimd.tensor_scalar(
                            out=JP[:], in0=PT[:, c, :], scalar1=0.5, scalar2=None,
                            op0=OP.is_gt, op1=OP.add, accum_out=CNT[:, c, n : n + 1],
                        )
                    else:
                        nc.scalar.activation(
                            out=JA[:], in_=PT[:, c, :], func=ACTF.Relu,
                            bias=NEGH[:, 0:1], scale=1.0,
                            accum_out=CNT[:, c, n : n + 1],
                        )
                # evacuate PSUM once on ACT (sum -> INSTP); DVE max reads the
                # cheaper SBUF copy
                PAIR = evacp.tile([2, W], F32, tag="pair")
                nc.scalar.activation(
                    out=PAIR[:], in_=PS[:], func=ACTF.Copy,
                    accum_out=INSTP[0:2, n : n + 1],
                )
                nc.vector.tensor_reduce(
                    out=HORP[0:2, n : n + 1], in_=PAIR[:], axis=AX.X, op=OP.max
                )

            # ---- vertical: occ = cnt > 0 ; vert = sum_h unit*occ ----
            OCC = consts.tile([128, NCH, N], F32)
            VERT = psv.tile([1, N], F32)
            for c in range(NCH):
                nc.vector.tensor_scalar(
                    out=OCC[:, c, :], in0=CNT[:, c, :], scalar1=0.0, scalar2=None,
                    op0=OP.is_gt,
                )
                nc.tensor.matmul(
                    out=VERT[:],
                    lhsT=UU[:, c, 0:1],
                    rhs=OCC[:, c, :],
                    start=(c == 0),
                    stop=(c == NCH - 1),
                )

            VERTS = consts.tile([1, N], F32)
            nc.scalar.copy(out=VERTS[:], in_=VERT[:])
            nc.sync.dma_start(out[0:1, :], INSTP[1:2, :])
            nc.sync.dma_start(out[1:2, :], HORP[0:1, :])
            nc.sync.dma_start(out[2:3, :], VERTS[:])


_NC = None


def _get_nc():
    global _NC
    if _NC is None:
        _NC = build_kernel()
    return _NC


def _consts():
    yf = (
        np.arange(128, dtype=np.float32)[:, None]
        + 128.0 * np.arange(NCH, dtype=np.float32)[None, :]
    ).copy()
    tril = np.triu(np.ones((128, 128), dtype=np.float32))  # [k,m] = 1 iff k<=m
    wv = np.arange(W, dtype=np.float32)
    amin4 = np.broadcast_to((W - wv).astype(np.float16), (128, NCH, W)).copy()
    amax4 = np.broadcast_to((wv + 1.0).astype(np.float16), (128, NCH, W)).copy()
    return yf, tril, amin4, amax4


def kernel(seg_outs: np.ndarray, pad_ins_outs: np.ndarray) -> np.ndarray:
    import ml_dtypes

    nc = _get_nc()
    yf, tril, amin4, amax4 = _consts()
    in_maps = []
    for b in range(B):
        seg_b = (
            seg_outs[b, :, :, 1]
            .reshape(NCH, 128, W)
            .transpose(1, 0, 2)
            .astype(ml_dtypes.bfloat16)
        )
        in_maps.append(
            {
                "seg": np.ascontiguousarray(seg_b),
                "pad": np.ascontiguousarray(pad_ins_outs[b]).astype(np.float16),
                "yf": yf,
                "tril": tril,
                "amin4": amin4,
                "amax4": amax4,
            }
        )
    res = run_bass_kernel_spmd(nc, in_maps, list(range(B)))
    outs = [res.results[b]["out"].T for b in range(B)]  # [N, 3] each
    return np.stack(outs, axis=0).astype(np.float32)


if __name__ == "__main__":
    rng = np.random.default_rng(0)
    seg_outs = rng.standard_normal((B, H, W, 2), dtype=np.float32)
    pad_ins_outs = rng.random((B, N, H, W), dtype=np.float32)
    print(kernel(seg_outs, pad_ins_outs)[0, :4])

